# revision 1
# baseline (speedup 1.0000x reference)
"""Multi-head attention (B=2, S=2048, D=1024, H=16, d_k=64) on 8 Trainium2
NeuronCores.

Sharding: core = b * 4 + g  (b = batch, g = head-group of 4 heads).
Each core projects Q/K/V for its 4 heads (column-sharded Wq/Wk/Wv), runs
masked softmax attention, and computes a partial output projection with the
row-shard of Wo.  The host sums the 4 partials per batch and adds bo.

Mask handling: the key mask is applied on the host by gathering only the
unmasked key columns (exp(-1e9) == 0 exactly in fp32, so the reference's
masked softmax is exactly a softmax over the unmasked keys).  Keys are
padded to a multiple of 128; pad keys get a zero "ones"-column entry and a
zero V row so they contribute nothing.

On-chip layouts (all contraction dims on SBUF partitions):
  qT/kT  [j, seq]   j = head*64 + dim, two tiles of 128 partitions
  V      [k, j]     natural layout per 128-key tile, with a kones column
                    per head so P@V_aug also yields the softmax denominator
  scores [k, q]     per key-tile in PSUM; softmax denom comes from the
                    kones column, so no cross-partition reductions needed
  attnT  [j, q]     feeds the output projection directly
Matmuls run in float32r (full-rate fp32, ~1.5e-4 rel err measured).
"""

import sys
import types

sys.path.insert(0, "/opt/trn_rl_repo")

# The axon boot script installs an NTFF-profiling hook only if
# ``antenv.axon_hooks`` is importable; this image's antenv lacks it, so
# provide it before anything triggers jax/axon platform registration.
if "antenv.axon_hooks" not in sys.modules:
    _hooks_mod = types.ModuleType("antenv.axon_hooks")
    _hooks_mod._hook = None

    def _set_hook(h, _m=_hooks_mod):
        _m._hook = h

    def _get_hook(_m=_hooks_mod):
        return _m._hook

    _hooks_mod.set_axon_ntff_profile_hook = _set_hook
    _hooks_mod.get_axon_ntff_profile_hook = _get_hook
    sys.modules["antenv.axon_hooks"] = _hooks_mod
    try:
        import antenv as _antenv

        _antenv.axon_hooks = _hooks_mod
    except ImportError:
        pass

import numpy as np

import concourse.bass as bass  # noqa: F401  (import keeps bass registered)
import concourse.mybir as mybir
import concourse.tile as tile
from concourse import bacc

F32 = mybir.dt.float32
F32R = mybir.dt.float32r
BF16 = mybir.dt.bfloat16
AF = mybir.ActivationFunctionType
ALU = mybir.AluOpType

D = 1024  # model dim
S = 2048  # sequence length
HL = 4  # heads per core
DK = 64  # head dim
J = HL * DK  # 256 projected dims per core
DC = D // 128  # 8 contraction chunks
JC = J // 128  # 2 j-chunks
B = 2
GROUPS = 4
NCORES = B * GROUPS

# per-head offsets inside a V stationary tile [128, 386]:
#   even heads: [V(64) | kones]              -> psum rows 0..63 data, 64 denom
#   odd heads:  [zeros(63) | kones | V(64)]  -> psum rows 64..127 data, 63 denom
V_REGION = [(0, 65), (65, 128), (193, 65), (258, 128)]  # (start, width)
V_DATA = [0, 129, 193, 322]  # start col of the 64 V columns
V_ONES = [64, 97, 257, 290]  # kones col (odd heads: region col 32 -> psum row 32)
V_WIDTH = 386


def build_program(kt_tiles: int):
    k_pad = kt_tiles * 128
    nc = bacc.Bacc()

    xq = nc.declare_dram_parameter("xq", [D, S], F32, isOutput=False)
    xk = nc.declare_dram_parameter("xk", [D, k_pad], F32, isOutput=False)
    xv = nc.declare_dram_parameter("xv", [D, k_pad], F32, isOutput=False)
    wq = nc.declare_dram_parameter("wq", [D, J], F32, isOutput=False)
    wk = nc.declare_dram_parameter("wk", [D, J], F32, isOutput=False)
    wv = nc.declare_dram_parameter("wv", [D, J], F32, isOutput=False)
    wo = nc.declare_dram_parameter("wo", [J, D], F32, isOutput=False)
    bq = nc.declare_dram_parameter("bq", [J], F32, isOutput=False)
    bk = nc.declare_dram_parameter("bk", [J], F32, isOutput=False)
    bv = nc.declare_dram_parameter("bv", [J], F32, isOutput=False)
    kones = nc.declare_dram_parameter("kones", [k_pad], F32, isOutput=False)
    y = nc.declare_dram_parameter("y", [S, D], F32, isOutput=True)

    r = lambda ap: ap.bitcast(F32R)

    with tile.TileContext(nc) as tc:
        with (
            tc.tile_pool(name="const", bufs=1) as cpool,
            tc.tile_pool(name="big", bufs=1) as big,
            tc.tile_pool(name="xin", bufs=3) as xin,
            tc.tile_pool(name="ptile", bufs=3) as ppool,
            tc.tile_pool(name="small", bufs=3) as small,
        ):
            # persistent activations (attention operands in bf16)
            qt_sb = [big.tile([128, S], BF16, tag=f"qt{jc}", name=f"qt{jc}") for jc in range(JC)]
            kt_sb = [big.tile([128, k_pad], BF16, tag=f"kt{h}", name=f"kt{h}") for h in range(HL)]
            # zero the unused half of each per-head K tile once: scores then
            # contract over K=128 (full rate) with the pad rows contributing 0
            for h in range(HL):
                po0 = 64 if h % 2 == 0 else 0
                nc.vector.memset(kt_sb[h][po0 : po0 + 64, :], 0.0)
            at_sb = [big.tile([128, S], F32R, tag=f"at{jc}", name=f"at{jc}") for jc in range(JC)]
            v_sb = [big.tile([128, V_WIDTH], BF16, tag=f"v{kt}", name=f"v{kt}") for kt in range(kt_tiles)]

            with tc.tile_pool(name="proj_psum", bufs=1, space="PSUM") as pp:
                # ---- Q^T projection (weights + x stream emitted just in time)
                wq_sb = cpool.tile([128, DC, J], F32R, tag="wq")
                nc.sync.dma_start(wq_sb[:], wq.rearrange("(c p) j -> p c j", p=128).bitcast(F32R))
                bq_sb = cpool.tile([128, JC], F32, tag="bq")
                nc.sync.dma_start(bq_sb[:], bq.rearrange("(c p) -> p c", p=128))

                QQC = S // 512
                psq = [pp.tile([128, 512], F32, tag=f"psq{i}", name=f"psq{i}") for i in range(JC * QQC)]
                for dc in range(DC):
                    xq_t = xin.tile([128, S], F32R, tag="xq")
                    nc.sync.dma_start(xq_t[:], xq[dc * 128 : (dc + 1) * 128, :].bitcast(F32R))
                    for jc in range(JC):
                        lhsT = wq_sb[:, dc, jc * 128 : (jc + 1) * 128]
                        for qc in range(QQC):
                            nc.tensor.matmul(
                                psq[jc * QQC + qc][:],
                                lhsT,
                                xq_t[:, qc * 512 : (qc + 1) * 512],
                                start=(dc == 0),
                                stop=(dc == DC - 1),
                            )
                for jc in range(JC):
                    for qc in range(QQC):
                        nc.vector.tensor_tensor(
                            qt_sb[jc][:, qc * 512 : (qc + 1) * 512],
                            psq[jc * QQC + qc][:],
                            bq_sb[:, jc : jc + 1].to_broadcast((128, 512)),
                            ALU.add,
                        )

                # ---- K^T projection
                wk_sb = cpool.tile([128, DC, J], F32R, tag="wk")
                nc.sync.dma_start(wk_sb[:], wk.rearrange("(c p) j -> p c j", p=128).bitcast(F32R))
                bk_sb = cpool.tile([128, JC], F32, tag="bk")
                nc.sync.dma_start(bk_sb[:], bk.rearrange("(c p) -> p c", p=128))

                kchunks = []
                off = 0
                while off < k_pad:
                    w = min(512, k_pad - off)
                    kchunks.append((off, w))
                    off += w
                psk = [
                    pp.tile([128, 512], F32, tag=f"psq{i}", name=f"psk{i}")
                    for i in range(JC * len(kchunks))
                ]
                for dc in range(DC):
                    xk_t = xin.tile([128, k_pad], F32R, tag="xk")
                    nc.sync.dma_start(xk_t[:], xk[dc * 128 : (dc + 1) * 128, :].bitcast(F32R))
                    for jc in range(JC):
                        lhsT = wk_sb[:, dc, jc * 128 : (jc + 1) * 128]
                        for i, (off, w) in enumerate(kchunks):
                            nc.tensor.matmul(
                                psk[jc * len(kchunks) + i][:, :w],
                                lhsT,
                                xk_t[:, off : off + w],
                                start=(dc == 0),
                                stop=(dc == DC - 1),
                            )
                for jc in range(JC):
                    for i, (off, w) in enumerate(kchunks):
                        ps_ = psk[jc * len(kchunks) + i]
                        nc.vector.tensor_tensor(
                            kt_sb[2 * jc][0:64, off : off + w],
                            ps_[0:64, :w],
                            bk_sb[0:64, jc : jc + 1].to_broadcast((64, w)),
                            ALU.add,
                        )
                        nc.vector.tensor_tensor(
                            kt_sb[2 * jc + 1][64:128, off : off + w],
                            ps_[64:128, :w],
                            bk_sb[64:128, jc : jc + 1].to_broadcast((64, w)),
                            ALU.add,
                        )

                # ---- V natural projection (+ kones columns) ----------------
                wv_sb = cpool.tile([128, DC, J], F32R, tag="wv")
                nc.sync.dma_start(wv_sb[:], wv.rearrange("(c p) j -> p c j", p=128).bitcast(F32R))
                xv_sb = big.tile([128, DC, k_pad], F32R, tag="xv")
                nc.sync.dma_start(xv_sb[:], xv.rearrange("(c p) k -> p c k", p=128).bitcast(F32R))
                bv_bc = cpool.tile([128, J], F32, tag="bv")
                nc.sync.dma_start(bv_bc[:], bv.ap()[None, :].to_broadcast((128, J)))
                kones_sb = cpool.tile([128, kt_tiles], F32, tag="kones")
                nc.sync.dma_start(kones_sb[:], kones.rearrange("(t p) -> p t", p=128))
                z64 = cpool.tile([128, 64], BF16, tag="z64")
                nc.vector.memset(z64[:], 0.0)
                wo_sb = cpool.tile([128, JC, D], F32R, tag="wo")
                nc.sync.dma_start(wo_sb[:], wo.rearrange("(c p) m -> p c m", p=128).bitcast(F32R))

                for kt in range(kt_tiles):
                    psv = pp.tile([128, J], F32, tag=f"psq{kt % 2}", name="psv")
                    for dc in range(DC):
                        nc.tensor.matmul(
                            psv[:],
                            xv_sb[:, dc, kt * 128 : (kt + 1) * 128],
                            wv_sb[:, dc, :],
                            start=(dc == 0),
                            stop=(dc == DC - 1),
                        )
                    vt = v_sb[kt]
                    nc.vector.tensor_copy(vt[:, 65:129], z64[:])
                    nc.vector.tensor_copy(vt[:, 258:322], z64[:])
                    kcol = kones_sb[:, kt : kt + 1]
                    for h in range(HL):
                        d0 = V_DATA[h]
                        nc.vector.tensor_tensor(
                            vt[:, d0 : d0 + DK],
                            psv[:, h * DK : (h + 1) * DK],
                            bv_bc[:, h * DK : (h + 1) * DK],
                            ALU.add,
                        )
                        nc.vector.tensor_scalar(
                            vt[:, d0 : d0 + DK],
                            vt[:, d0 : d0 + DK],
                            kcol,
                            None,
                            ALU.mult,
                        )
                        nc.vector.tensor_copy(vt[:, V_ONES[h] : V_ONES[h] + 1], kcol)

            # ---- attention + interleaved output projection -----------------
            QH = S // 1024  # 2 q halves
            with (
                tc.tile_pool(name="score_psum", bufs=2, space="PSUM") as sp,
                tc.tile_pool(name="aug_psum", bufs=2, space="PSUM") as ap,
            ):
                for qh in range(QH):
                    for h in range(HL):
                        jc = h // 2
                        po = (h % 2) * 64  # partition offset of this head's data
                        den = 64 if h % 2 == 0 else 32
                        vstart, vwidth = V_REGION[h]
                        aug = ap.tile([128, 1024], F32, tag="aug", name="aug")
                        for kt in range(kt_tiles):
                            ps = sp.tile([128, 1024], F32, tag="ps", name="ps")
                            lhs_k = kt_sb[h][:, kt * 128 : (kt + 1) * 128]
                            for i in range(2):
                                nc.tensor.matmul(
                                    ps[:, i * 512 : (i + 1) * 512],
                                    lhs_k,
                                    qt_sb[jc][
                                        :,
                                        qh * 1024 + i * 512 : qh * 1024 + (i + 1) * 512,
                                    ],
                                    start=True,
                                    stop=True,
                                )
                            pt = ppool.tile([128, 1024], BF16, tag="pt")
                            nc.scalar.activation(pt[:], ps[:], AF.Exp, scale=0.125)
                            lhs_v = v_sb[kt][:, vstart : vstart + vwidth]
                            for i in range(2):
                                nc.tensor.matmul(
                                    aug[:vwidth, i * 512 : (i + 1) * 512],
                                    lhs_v,
                                    pt[:, i * 512 : (i + 1) * 512],
                                    start=(kt == 0),
                                    stop=(kt == kt_tiles - 1),
                                )
                        dt_ = small.tile([1, 1024], F32, tag="dt")
                        nc.scalar.copy(dt_[:], aug[den : den + 1, :])
                        rt = small.tile([1, 1024], F32, tag="rt")
                        nc.vector.reciprocal_approx_fast(rt[:], dt_[:])
                        rb = small.tile([128, 1024], F32, tag="rb")
                        nc.gpsimd.partition_broadcast(rb[:], rt[:])
                        nc.vector.tensor_tensor(
                            at_sb[jc][po : po + DK, qh * 1024 : (qh + 1) * 1024],
                            aug[po : po + DK, :],
                            rb[po : po + DK, :],
                            ALU.mult,
                        )

                    # output projection for this q half (psum slots shared
                    # with the score tiles via the "ps" tag)
                    for qt in range(qh * 8, (qh + 1) * 8):
                        psy = sp.tile([128, 1024], F32, tag="ps", name="psy")
                        for jc in range(JC):
                            lhsT = at_sb[jc][:, qt * 128 : (qt + 1) * 128]
                            for mc in range(2):
                                nc.tensor.matmul(
                                    psy[:, mc * 512 : (mc + 1) * 512],
                                    lhsT,
                                    wo_sb[:, jc, mc * 512 : (mc + 1) * 512],
                                    start=(jc == 0),
                                    stop=(jc == JC - 1),
                                )
                        yt = small.tile([128, 1024], F32, tag="yt", name="yt")
                        nc.vector.tensor_copy(yt[:], psy[:])
                        nc.sync.dma_start(y[qt * 128 : (qt + 1) * 128, :], yt[:])

    nc.finalize()
    return nc


_CACHE: dict = {}


def _get_program(kt_tiles: int):
    if kt_tiles not in _CACHE:
        _CACHE[kt_tiles] = build_program(kt_tiles)
    return _CACHE[kt_tiles]


def _prep_inputs(q, k, v, mask, Wq, bq, Wk, bk, Wv, bv, Wo, bo):
    """Shard + transpose + compact on the host. Returns (in_maps, kt_tiles)."""
    idx = [np.nonzero(mask[b])[0] for b in range(B)]
    s_u = max(1, max(len(i) for i in idx))
    kt_tiles = (s_u + 127) // 128
    k_pad = kt_tiles * 128

    per_batch = []
    for b in range(B):
        qT = np.ascontiguousarray(q[b].T)  # [D, S]
        kT = np.zeros((D, k_pad), np.float32)
        vT = np.zeros((D, k_pad), np.float32)
        n = len(idx[b])
        kT[:, :n] = k[b].T[:, idx[b]]
        vT[:, :n] = v[b].T[:, idx[b]]
        ko = np.zeros((k_pad,), np.float32)
        ko[:n] = 1.0
        per_batch.append((qT, kT, vT, ko))

    in_maps = []
    for core in range(NCORES):
        b, g = divmod(core, GROUPS)
        j0 = g * J
        qT, kT, vT, ko = per_batch[b]
        in_maps.append(
            {
                "xq": qT,
                "xk": kT,
                "xv": vT,
                "wq": np.ascontiguousarray(Wq[j0 : j0 + J, :].T),
                "wk": np.ascontiguousarray(Wk[j0 : j0 + J, :].T),
                "wv": np.ascontiguousarray(Wv[j0 : j0 + J, :].T),
                "wo": np.ascontiguousarray(Wo[:, j0 : j0 + J].T),
                "bq": np.ascontiguousarray(bq[j0 : j0 + J]),
                "bk": np.ascontiguousarray(bk[j0 : j0 + J]),
                "bv": np.ascontiguousarray(bv[j0 : j0 + J]),
                "kones": ko,
            }
        )
    return in_maps, kt_tiles


def run(inputs: dict, trace: bool = False):
    """Run the sharded kernel; returns (output [B,S,D] f32, BassKernelResults)."""
    from concourse.bass_utils import run_bass_kernel_spmd

    inputs = {k: np.asarray(v) for k, v in inputs.items()}
    in_maps, kt_tiles = _prep_inputs(**inputs)
    nc = _get_program(kt_tiles)
    res = run_bass_kernel_spmd(nc, in_maps, list(range(NCORES)), trace=trace)
    bo = inputs["bo"].astype(np.float32)
    out = np.empty((B, S, D), np.float32)
    for b in range(B):
        acc = bo[None, :].astype(np.float64) * 0.0
        acc = np.zeros((S, D), np.float64)
        for g in range(GROUPS):
            acc += res.results[b * GROUPS + g]["y"]
        out[b] = (acc + bo[None, :]).astype(np.float32)
    return out, res


def kernel(**inputs) -> np.ndarray:
    out, _ = run(inputs, trace=False)
    return out



# revision 5
# speedup vs baseline: 1.1613x; 1.1613x over previous
"""Multi-head attention (B=2, S=2048, D=1024, H=16, d_k=64) on 8 Trainium2
NeuronCores.

Sharding: core = b * 4 + g  (b = batch, g = head-group of 4 heads).
Each core projects Q/K/V for its 4 heads (column-sharded Wq/Wk/Wv), runs
masked softmax attention, and computes a partial output projection with the
row-shard of Wo.  The host sums the 4 partials per batch and adds bo.

Mask handling: the key mask is applied on the host by gathering only the
unmasked key columns (exp(-1e9) == 0 exactly in fp32, so the reference's
masked softmax is exactly a softmax over the unmasked keys).  Keys are
padded to a multiple of 128; pad keys get zero "ones" columns and zero V
rows so they contribute nothing.

v2 layout (vs the f32 baseline):
  * all DRAM traffic in bf16 (x, weights, y partials) — halves DMA time and
    enables FWL fast weight loads on every matmul.
  * K kept merged per j-chunk: kt2[jc] [128, k_pad] holds head 2jc on
    partitions 0:64 and head 2jc+1 on 64:128.  Score matmuls contract over
    64 partitions and run as row-tiled pairs (tile_position rows 0 / 64)
    concurrently in the PE array.
  * V stationary tiles carry a 64-wide ones block per head, so the PV
    matmul replicates the softmax denominator across 64 PSUM partitions:
      even head: [V(64) | ones(64)]  -> aug rows 0:64 data, 64:128 denom
      odd head:  [ones(64) | V(64)]  -> aug rows 0:64 denom, 64:128 data
    Normalize is then two plain DVE ops (reciprocal with a cross-half
    write, multiply) — no ScalarE copies, no GpSimd broadcasts.  ScalarE
    does nothing but Exp.
  * attention runs per 512-wide q granule; the output projection for a
    granule is emitted as soon as its 4 heads finish, so y DMA dribbles
    out through the whole attention phase.
"""

import sys
import types

sys.path.insert(0, "/opt/trn_rl_repo")

# The axon boot script installs an NTFF-profiling hook only if
# ``antenv.axon_hooks`` is importable; this image's antenv lacks it, so
# provide it before anything triggers jax/axon platform registration.
if "antenv.axon_hooks" not in sys.modules:
    _hooks_mod = types.ModuleType("antenv.axon_hooks")
    _hooks_mod._hook = None

    def _set_hook(h, _m=_hooks_mod):
        _m._hook = h

    def _get_hook(_m=_hooks_mod):
        return _m._hook

    _hooks_mod.set_axon_ntff_profile_hook = _set_hook
    _hooks_mod.get_axon_ntff_profile_hook = _get_hook
    sys.modules["antenv.axon_hooks"] = _hooks_mod
    try:
        import antenv as _antenv

        _antenv.axon_hooks = _hooks_mod
    except ImportError:
        pass

import ml_dtypes
import numpy as np

import concourse.bass as bass  # noqa: F401  (import keeps bass registered)
import concourse.mybir as mybir
import concourse.tile as tile
from concourse import bacc

F32 = mybir.dt.float32
BF16 = mybir.dt.bfloat16
AF = mybir.ActivationFunctionType
ALU = mybir.AluOpType
BF16NP = ml_dtypes.bfloat16

D = 1024  # model dim
S = 2048  # sequence length
HL = 4  # heads per core
DK = 64  # head dim
J = HL * DK  # 256 projected dims per core
DC = D // 128  # 8 contraction chunks
JC = J // 128  # 2 j-chunks
B = 2
GROUPS = 4
NCORES = B * GROUPS
QG = 512  # q granule width
NQG = S // QG


def build_program(kt_tiles: int):
    k_pad = kt_tiles * 128
    nc = bacc.Bacc()

    xq = nc.declare_dram_parameter("xq", [D, S], BF16, isOutput=False)
    xk = nc.declare_dram_parameter("xk", [D, k_pad], BF16, isOutput=False)
    xv = nc.declare_dram_parameter("xv", [D, k_pad], BF16, isOutput=False)
    wq = nc.declare_dram_parameter("wq", [D, J], BF16, isOutput=False)
    wk = nc.declare_dram_parameter("wk", [D, J], BF16, isOutput=False)
    wv = nc.declare_dram_parameter("wv", [D, J], BF16, isOutput=False)
    wo = nc.declare_dram_parameter("wo", [J, D], BF16, isOutput=False)
    bq = nc.declare_dram_parameter("bq", [J], F32, isOutput=False)
    bk = nc.declare_dram_parameter("bk", [J], F32, isOutput=False)
    bv = nc.declare_dram_parameter("bv", [J], F32, isOutput=False)
    kones = nc.declare_dram_parameter("kones", [k_pad], F32, isOutput=False)
    y = nc.declare_dram_parameter("y", [S, D], BF16, isOutput=True)

    with tile.TileContext(nc) as tc:
        with (
            tc.tile_pool(name="const", bufs=1) as cpool,
            tc.tile_pool(name="big", bufs=1) as big,
            tc.tile_pool(name="xin", bufs=3) as xin,
            tc.tile_pool(name="ptile", bufs=3) as ppool,
            tc.tile_pool(name="small", bufs=3) as small,
        ):
            # persistent activations (all bf16)
            qt_sb = [big.tile([128, S], BF16, tag=f"qt{jc}", name=f"qt{jc}") for jc in range(JC)]
            kt2 = [big.tile([128, k_pad], BF16, tag=f"kt{jc}", name=f"kt{jc}") for jc in range(JC)]
            at_sb = [big.tile([128, S], BF16, tag=f"at{jc}", name=f"at{jc}") for jc in range(JC)]
            v_sb = [big.tile([128, 512], BF16, tag=f"v{kt}", name=f"v{kt}") for kt in range(kt_tiles)]

            with tc.tile_pool(name="proj_psum", bufs=1, space="PSUM") as pp:
                # ---- Q^T projection (weights + x stream emitted just in time)
                wq_sb = cpool.tile([128, DC, J], BF16, tag="wq")
                nc.sync.dma_start(wq_sb[:], wq.rearrange("(c p) j -> p c j", p=128))
                bq_sb = cpool.tile([128, JC], F32, tag="bq")
                nc.sync.dma_start(bq_sb[:], bq.rearrange("(c p) -> p c", p=128))

                QQC = S // 512
                psq = [pp.tile([128, 512], F32, tag=f"psq{i}", name=f"psq{i}") for i in range(JC * QQC)]
                for dc in range(DC):
                    xq_t = xin.tile([128, S], BF16, tag="xq")
                    nc.sync.dma_start(xq_t[:], xq[dc * 128 : (dc + 1) * 128, :])
                    for jc in range(JC):
                        lhsT = wq_sb[:, dc, jc * 128 : (jc + 1) * 128]
                        for qc in range(QQC):
                            nc.tensor.matmul(
                                psq[jc * QQC + qc][:],
                                lhsT,
                                xq_t[:, qc * 512 : (qc + 1) * 512],
                                start=(dc == 0),
                                stop=(dc == DC - 1),
                            )
                for jc in range(JC):
                    for qc in range(QQC):
                        nc.vector.tensor_tensor(
                            qt_sb[jc][:, qc * 512 : (qc + 1) * 512],
                            psq[jc * QQC + qc][:],
                            bq_sb[:, jc : jc + 1].to_broadcast((128, 512)),
                            ALU.add,
                        )

                # ---- K^T projection (merged per-chunk layout)
                wk_sb = cpool.tile([128, DC, J], BF16, tag="wk")
                nc.sync.dma_start(wk_sb[:], wk.rearrange("(c p) j -> p c j", p=128))
                bk_sb = cpool.tile([128, JC], F32, tag="bk")
                nc.sync.dma_start(bk_sb[:], bk.rearrange("(c p) -> p c", p=128))

                kchunks = []
                off = 0
                while off < k_pad:
                    w = min(512, k_pad - off)
                    kchunks.append((off, w))
                    off += w
                psk = [
                    pp.tile([128, 512], F32, tag=f"psq{i}", name=f"psk{i}")
                    for i in range(JC * len(kchunks))
                ]
                for dc in range(DC):
                    xk_t = xin.tile([128, k_pad], BF16, tag="xk")
                    nc.sync.dma_start(xk_t[:], xk[dc * 128 : (dc + 1) * 128, :])
                    for jc in range(JC):
                        lhsT = wk_sb[:, dc, jc * 128 : (jc + 1) * 128]
                        for i, (off, w) in enumerate(kchunks):
                            nc.tensor.matmul(
                                psk[jc * len(kchunks) + i][:, :w],
                                lhsT,
                                xk_t[:, off : off + w],
                                start=(dc == 0),
                                stop=(dc == DC - 1),
                            )
                for jc in range(JC):
                    for i, (off, w) in enumerate(kchunks):
                        nc.vector.tensor_tensor(
                            kt2[jc][:, off : off + w],
                            psk[jc * len(kchunks) + i][:, :w],
                            bk_sb[:, jc : jc + 1].to_broadcast((128, w)),
                            ALU.add,
                        )

                # ---- V natural projection (+ per-head 64-wide ones blocks) --
                wv_sb = cpool.tile([128, DC, J], BF16, tag="wv")
                nc.sync.dma_start(wv_sb[:], wv.rearrange("(c p) j -> p c j", p=128))
                xv_sb = big.tile([128, DC, k_pad], BF16, tag="xv")
                nc.sync.dma_start(xv_sb[:], xv.rearrange("(c p) k -> p c k", p=128))
                bv_bc = cpool.tile([128, J], F32, tag="bv")
                nc.sync.dma_start(bv_bc[:], bv.ap()[None, :].to_broadcast((128, J)))
                kones_sb = cpool.tile([128, kt_tiles], F32, tag="kones")
                nc.sync.dma_start(kones_sb[:], kones.rearrange("(t p) -> p t", p=128))
                wo_sb = cpool.tile([128, JC, D], BF16, tag="wo")
                nc.sync.dma_start(wo_sb[:], wo.rearrange("(c p) m -> p c m", p=128))

                for kt in range(kt_tiles):
                    psv = pp.tile([128, J], F32, tag=f"psq{kt % 2}", name="psv")
                    for dc in range(DC):
                        nc.tensor.matmul(
                            psv[:],
                            xv_sb[:, dc, kt * 128 : (kt + 1) * 128],
                            wv_sb[:, dc, :],
                            start=(dc == 0),
                            stop=(dc == DC - 1),
                        )
                    vt = v_sb[kt]
                    kcol = kones_sb[:, kt : kt + 1]
                    for h in range(HL):
                        pair = h // 2
                        if h % 2 == 0:
                            d0 = pair * 256
                            o0 = pair * 256 + 64
                        else:
                            o0 = pair * 256 + 128
                            d0 = pair * 256 + 192
                        nc.vector.tensor_tensor(
                            vt[:, d0 : d0 + DK],
                            psv[:, h * DK : (h + 1) * DK],
                            bv_bc[:, h * DK : (h + 1) * DK],
                            ALU.add,
                        )
                        nc.vector.tensor_scalar(
                            vt[:, d0 : d0 + DK],
                            vt[:, d0 : d0 + DK],
                            kcol,
                            None,
                            ALU.mult,
                        )
                        nc.vector.tensor_copy(
                            vt[:, o0 : o0 + DK], kcol.to_broadcast((128, DK))
                        )

            # ---- attention + per-granule output projection ------------------
            with (
                tc.tile_pool(name="score_psum", bufs=2, space="PSUM") as sp,
                tc.tile_pool(name="aug_psum", bufs=4, space="PSUM") as ap,
            ):
                for qg in range(NQG):
                    q0 = qg * QG
                    for pair in range(JC):
                        aug_e = ap.tile([128, QG], F32, tag="aug", name="aug_e")
                        aug_o = ap.tile([128, QG], F32, tag="aug", name="aug_o")
                        for kt in range(kt_tiles):
                            ps = sp.tile([128, 1024], F32, tag="ps", name="ps")
                            ksl = slice(kt * 128, (kt + 1) * 128)
                            # row-tiled concurrent score pair (K=64 each)
                            nc.tensor.matmul(
                                ps[:, 0:QG],
                                kt2[pair][0:64, ksl],
                                qt_sb[pair][0:64, q0 : q0 + QG],
                                start=True,
                                stop=True,
                            )
                            nc.tensor.matmul(
                                ps[:, QG : 2 * QG],
                                kt2[pair][64:128, ksl],
                                qt_sb[pair][64:128, q0 : q0 + QG],
                                start=True,
                                stop=True,
                            )
                            pt = ppool.tile([128, 1024], BF16, tag="pt")
                            nc.scalar.activation(pt[:], ps[:], AF.Exp, scale=0.125)
                            nc.tensor.matmul(
                                aug_e[:],
                                v_sb[kt][:, pair * 256 : pair * 256 + 128],
                                pt[:, 0:QG],
                                start=(kt == 0),
                                stop=(kt == kt_tiles - 1),
                            )
                            nc.tensor.matmul(
                                aug_o[:],
                                v_sb[kt][:, pair * 256 + 128 : pair * 256 + 256],
                                pt[:, QG : 2 * QG],
                                start=(kt == 0),
                                stop=(kt == kt_tiles - 1),
                            )
                        # normalize: the custom reciprocal op only works at
                        # base partition 0 on HW, and cross-half DVE moves
                        # are invalid — partition shifts go through small
                        # SBUF->SBUF DMAs instead.
                        de = small.tile([128, QG], F32, tag="de")
                        nc.vector.tensor_copy(de[64:128, :], aug_e[64:128, :])
                        dl = small.tile([128, QG], F32, tag="dl")
                        nc.sync.dma_start(dl[0:64, :], de[64:128, :])
                        rr = small.tile([128, QG], F32, tag="rr")
                        nc.vector.reciprocal_approx_fast(rr[0:64, :], dl[0:64, :])
                        ro = small.tile([128, QG], F32, tag="ro")
                        nc.vector.reciprocal_approx_fast(ro[0:64, :], aug_o[0:64, :])
                        rb = small.tile([128, QG], F32, tag="rb")
                        nc.sync.dma_start(rb[0:64, :], rr[0:64, :])
                        nc.sync.dma_start(rb[64:128, :], ro[0:64, :])
                        nc.vector.tensor_tensor(
                            at_sb[pair][0:64, q0 : q0 + QG],
                            aug_e[0:64, :],
                            rb[0:64, :],
                            ALU.mult,
                        )
                        nc.vector.tensor_tensor(
                            at_sb[pair][64:128, q0 : q0 + QG],
                            aug_o[64:128, :],
                            rb[64:128, :],
                            ALU.mult,
                        )

                    # output projection for this q granule (psum slots shared
                    # with the score tiles via the "ps" tag)
                    for qt in range(qg * 4, (qg + 1) * 4):
                        psy = sp.tile([128, 1024], F32, tag="ps", name="psy")
                        for jc in range(JC):
                            lhsT = at_sb[jc][:, qt * 128 : (qt + 1) * 128]
                            for mc in range(2):
                                nc.tensor.matmul(
                                    psy[:, mc * 512 : (mc + 1) * 512],
                                    lhsT,
                                    wo_sb[:, jc, mc * 512 : (mc + 1) * 512],
                                    start=(jc == 0),
                                    stop=(jc == JC - 1),
                                )
                        yt = small.tile([128, 1024], BF16, tag="yt", name="yt")
                        nc.vector.tensor_copy(yt[:], psy[:])
                        nc.sync.dma_start(y[qt * 128 : (qt + 1) * 128, :], yt[:])

    nc.finalize()
    return nc


_CACHE: dict = {}


def _get_program(kt_tiles: int):
    if kt_tiles not in _CACHE:
        _CACHE[kt_tiles] = build_program(kt_tiles)
    return _CACHE[kt_tiles]


def _prep_inputs(q, k, v, mask, Wq, bq, Wk, bk, Wv, bv, Wo, bo):
    """Shard + transpose + compact on the host. Returns (in_maps, kt_tiles)."""
    idx = [np.nonzero(mask[b])[0] for b in range(B)]
    s_u = max(1, max(len(i) for i in idx))
    kt_tiles = (s_u + 127) // 128
    k_pad = kt_tiles * 128

    per_batch = []
    for b in range(B):
        qT = np.ascontiguousarray(q[b].T).astype(BF16NP)  # [D, S]
        kT = np.zeros((D, k_pad), BF16NP)
        vT = np.zeros((D, k_pad), BF16NP)
        n = len(idx[b])
        kT[:, :n] = k[b].T[:, idx[b]].astype(BF16NP)
        vT[:, :n] = v[b].T[:, idx[b]].astype(BF16NP)
        ko = np.zeros((k_pad,), np.float32)
        ko[:n] = 1.0
        per_batch.append((qT, kT, vT, ko))

    in_maps = []
    for core in range(NCORES):
        b, g = divmod(core, GROUPS)
        j0 = g * J
        qT, kT, vT, ko = per_batch[b]
        in_maps.append(
            {
                "xq": qT,
                "xk": kT,
                "xv": vT,
                "wq": np.ascontiguousarray(Wq[j0 : j0 + J, :].T).astype(BF16NP),
                "wk": np.ascontiguousarray(Wk[j0 : j0 + J, :].T).astype(BF16NP),
                "wv": np.ascontiguousarray(Wv[j0 : j0 + J, :].T).astype(BF16NP),
                "wo": np.ascontiguousarray(Wo[:, j0 : j0 + J].T).astype(BF16NP),
                "bq": np.ascontiguousarray(bq[j0 : j0 + J]).astype(np.float32),
                "bk": np.ascontiguousarray(bk[j0 : j0 + J]).astype(np.float32),
                "bv": np.ascontiguousarray(bv[j0 : j0 + J]).astype(np.float32),
                "kones": ko,
            }
        )
    return in_maps, kt_tiles


def run(inputs: dict, trace: bool = False):
    """Run the sharded kernel; returns (output [B,S,D] f32, BassKernelResults)."""
    from concourse.bass_utils import run_bass_kernel_spmd

    inputs = {k: np.asarray(v) for k, v in inputs.items()}
    in_maps, kt_tiles = _prep_inputs(**inputs)
    nc = _get_program(kt_tiles)
    res = run_bass_kernel_spmd(nc, in_maps, list(range(NCORES)), trace=trace)
    bo = inputs["bo"].astype(np.float32)
    out = np.empty((B, S, D), np.float32)
    for b in range(B):
        acc = np.zeros((S, D), np.float64)
        for g in range(GROUPS):
            acc += np.asarray(res.results[b * GROUPS + g]["y"], dtype=np.float64)
        out[b] = (acc + bo[None, :]).astype(np.float32)
    return out, res


def kernel(**inputs) -> np.ndarray:
    out, _ = run(inputs, trace=False)
    return out


# revision 10
# speedup vs baseline: 1.2215x; 1.0518x over previous
"""Multi-head attention (B=2, S=2048, D=1024, H=16, d_k=64) on 8 Trainium2
NeuronCores.

Sharding: core = b * 4 + g  (b = batch, g = head-group of 4 heads).
Each core projects Q/K/V for its 4 heads (column-sharded Wq/Wk/Wv), runs
masked softmax attention, and computes a partial output projection with the
row-shard of Wo.  The host sums the 4 partials per batch and adds bo.

Mask handling: the key mask is applied on the host by gathering only the
unmasked key columns (exp(-1e9) == 0 exactly in fp32, so the reference's
masked softmax is exactly a softmax over the unmasked keys).  Keys are
padded to a multiple of 128; pad keys get zero "ones" columns and zero V
rows so they contribute nothing.

v2 layout (vs the f32 baseline):
  * all DRAM traffic in bf16 (x, weights, y partials) — halves DMA time and
    enables FWL fast weight loads on every matmul.
  * K kept merged per j-chunk: kt2[jc] [128, k_pad] holds head 2jc on
    partitions 0:64 and head 2jc+1 on 64:128.  Score matmuls contract over
    64 partitions and run as row-tiled pairs (tile_position rows 0 / 64)
    concurrently in the PE array.
  * V stationary tiles carry a 64-wide ones block per head, so the PV
    matmul replicates the softmax denominator across 64 PSUM partitions:
      even head: [V(64) | ones(64)]  -> aug rows 0:64 data, 64:128 denom
      odd head:  [ones(64) | V(64)]  -> aug rows 0:64 denom, 64:128 data
    Normalize is then two plain DVE ops (reciprocal with a cross-half
    write, multiply) — no ScalarE copies, no GpSimd broadcasts.  ScalarE
    does nothing but Exp.
  * attention runs per 512-wide q granule; the output projection for a
    granule is emitted as soon as its 4 heads finish, so y DMA dribbles
    out through the whole attention phase.
"""

import sys
import types

sys.path.insert(0, "/opt/trn_rl_repo")

# The axon boot script installs an NTFF-profiling hook only if
# ``antenv.axon_hooks`` is importable; this image's antenv lacks it, so
# provide it before anything triggers jax/axon platform registration.
if "antenv.axon_hooks" not in sys.modules:
    _hooks_mod = types.ModuleType("antenv.axon_hooks")
    _hooks_mod._hook = None

    def _set_hook(h, _m=_hooks_mod):
        _m._hook = h

    def _get_hook(_m=_hooks_mod):
        return _m._hook

    _hooks_mod.set_axon_ntff_profile_hook = _set_hook
    _hooks_mod.get_axon_ntff_profile_hook = _get_hook
    sys.modules["antenv.axon_hooks"] = _hooks_mod
    try:
        import antenv as _antenv

        _antenv.axon_hooks = _hooks_mod
    except ImportError:
        pass

import ml_dtypes
import numpy as np

import concourse.bass as bass  # noqa: F401  (import keeps bass registered)
import concourse.mybir as mybir
import concourse.tile as tile
from concourse import bacc

F32 = mybir.dt.float32
BF16 = mybir.dt.bfloat16
AF = mybir.ActivationFunctionType
ALU = mybir.AluOpType
BF16NP = ml_dtypes.bfloat16

D = 1024  # model dim
S = 2048  # sequence length
HL = 4  # heads per core
DK = 64  # head dim
J = HL * DK  # 256 projected dims per core
DC = D // 128  # 8 contraction chunks
JC = J // 128  # 2 j-chunks
B = 2
GROUPS = 4
NCORES = B * GROUPS
QG = 512  # q granule width
NQG = S // QG


def emit_outproj(nc, sp, small, at_sb, wo_sb, y, qg):
    """Emit the output projection for q granule qg (4 row tiles of 128)."""
    for qt in range(qg * 4, (qg + 1) * 4):
        psy = sp.tile([128, 1024], F32, tag="ps", name="psy")
        for jc in range(JC):
            lhsT = at_sb[jc][:, qt * 128 : (qt + 1) * 128]
            for mc in range(2):
                nc.tensor.matmul(
                    psy[:, mc * 512 : (mc + 1) * 512],
                    lhsT,
                    wo_sb[:, jc, mc * 512 : (mc + 1) * 512],
                    start=(jc == 0),
                    stop=(jc == JC - 1),
                )
        yt = small.tile([128, 1024], BF16, tag="yt", name="yt")
        nc.vector.tensor_copy(yt[:], psy[:])
        nc.sync.dma_start(y[qt * 128 : (qt + 1) * 128, :], yt[:])


def build_program(kt_tiles: int):
    k_pad = kt_tiles * 128
    nc = bacc.Bacc()

    xq = nc.declare_dram_parameter("xq", [D, S], BF16, isOutput=False)
    xk = nc.declare_dram_parameter("xk", [D, k_pad], BF16, isOutput=False)
    xv = nc.declare_dram_parameter("xv", [D, k_pad], BF16, isOutput=False)
    wq = nc.declare_dram_parameter("wq", [D, J], BF16, isOutput=False)
    wk = nc.declare_dram_parameter("wk", [D, J], BF16, isOutput=False)
    wv = nc.declare_dram_parameter("wv", [D, J], BF16, isOutput=False)
    wo = nc.declare_dram_parameter("wo", [J, D], BF16, isOutput=False)
    bq = nc.declare_dram_parameter("bq", [J], F32, isOutput=False)
    bk = nc.declare_dram_parameter("bk", [J], F32, isOutput=False)
    bv = nc.declare_dram_parameter("bv", [J], F32, isOutput=False)
    kones = nc.declare_dram_parameter("kones", [k_pad], F32, isOutput=False)
    y = nc.declare_dram_parameter("y", [S, D], BF16, isOutput=True)

    with tile.TileContext(nc) as tc:
        with (
            tc.tile_pool(name="const", bufs=1) as cpool,
            tc.tile_pool(name="big", bufs=1) as big,
            tc.tile_pool(name="xin", bufs=3) as xin,
            tc.tile_pool(name="ptile", bufs=3) as ppool,
            tc.tile_pool(name="small", bufs=3) as small,
        ):
            # persistent activations (all bf16)
            qt_sb = [big.tile([128, S], BF16, tag=f"qt{jc}", name=f"qt{jc}") for jc in range(JC)]
            kt2 = [big.tile([128, k_pad], BF16, tag=f"kt{jc}", name=f"kt{jc}") for jc in range(JC)]
            at_sb = [big.tile([128, S], BF16, tag=f"at{jc}", name=f"at{jc}") for jc in range(JC)]
            v_sb = [big.tile([128, 512], BF16, tag=f"v{kt}", name=f"v{kt}") for kt in range(kt_tiles)]

            with tc.tile_pool(name="proj_psum", bufs=1, space="PSUM") as pp:
                # issue every weight/constant DMA up front so the later
                # projection stages never wait on cold transfers
                wq_sb = cpool.tile([128, DC, J], BF16, tag="wq")
                nc.sync.dma_start(wq_sb[:], wq.rearrange("(c p) j -> p c j", p=128))
                bq_sb = cpool.tile([128, JC], F32, tag="bq")
                nc.sync.dma_start(bq_sb[:], bq.rearrange("(c p) -> p c", p=128))
                wk_sb = cpool.tile([128, DC, J], BF16, tag="wk")
                nc.sync.dma_start(wk_sb[:], wk.rearrange("(c p) j -> p c j", p=128))
                bk_sb = cpool.tile([128, JC], F32, tag="bk")
                nc.sync.dma_start(bk_sb[:], bk.rearrange("(c p) -> p c", p=128))
                wv_sb = cpool.tile([128, DC, J], BF16, tag="wv")
                nc.sync.dma_start(wv_sb[:], wv.rearrange("(c p) j -> p c j", p=128))
                xv_sb = big.tile([128, DC, k_pad], BF16, tag="xv")
                nc.sync.dma_start(xv_sb[:], xv.rearrange("(c p) k -> p c k", p=128))
                bv_bc = cpool.tile([128, J], F32, tag="bv")
                nc.sync.dma_start(bv_bc[:], bv.ap()[None, :].to_broadcast((128, J)))
                kones_sb = cpool.tile([128, kt_tiles], F32, tag="kones")
                nc.sync.dma_start(kones_sb[:], kones.rearrange("(t p) -> p t", p=128))
                wo_sb = cpool.tile([128, JC, D], BF16, tag="wo")
                nc.sync.dma_start(wo_sb[:], wo.rearrange("(c p) m -> p c m", p=128))

                # ---- Q^T projection (x stream emitted just in time)

                QQC = S // 512
                psq = [pp.tile([128, 512], F32, tag=f"psq{i}", name=f"psq{i}") for i in range(JC * QQC)]
                for dc in range(DC):
                    xq_t = xin.tile([128, S], BF16, tag="xq")
                    nc.sync.dma_start(xq_t[:], xq[dc * 128 : (dc + 1) * 128, :])
                    for jc in range(JC):
                        lhsT = wq_sb[:, dc, jc * 128 : (jc + 1) * 128]
                        for qc in range(QQC):
                            nc.tensor.matmul(
                                psq[jc * QQC + qc][:],
                                lhsT,
                                xq_t[:, qc * 512 : (qc + 1) * 512],
                                start=(dc == 0),
                                stop=(dc == DC - 1),
                            )
                for jc in range(JC):
                    for qc in range(QQC):
                        nc.vector.tensor_tensor(
                            qt_sb[jc][:, qc * 512 : (qc + 1) * 512],
                            psq[jc * QQC + qc][:],
                            bq_sb[:, jc : jc + 1].to_broadcast((128, 512)),
                            ALU.add,
                        )

                # ---- K^T projection (merged per-chunk layout)
                kchunks = []
                off = 0
                while off < k_pad:
                    w = min(512, k_pad - off)
                    kchunks.append((off, w))
                    off += w
                psk = [
                    pp.tile([128, 512], F32, tag=f"psq{i}", name=f"psk{i}")
                    for i in range(JC * len(kchunks))
                ]
                for dc in range(DC):
                    xk_t = xin.tile([128, k_pad], BF16, tag="xk")
                    nc.sync.dma_start(xk_t[:], xk[dc * 128 : (dc + 1) * 128, :])
                    for jc in range(JC):
                        lhsT = wk_sb[:, dc, jc * 128 : (jc + 1) * 128]
                        for i, (off, w) in enumerate(kchunks):
                            nc.tensor.matmul(
                                psk[jc * len(kchunks) + i][:, :w],
                                lhsT,
                                xk_t[:, off : off + w],
                                start=(dc == 0),
                                stop=(dc == DC - 1),
                            )
                for jc in range(JC):
                    for i, (off, w) in enumerate(kchunks):
                        nc.vector.tensor_tensor(
                            kt2[jc][:, off : off + w],
                            psk[jc * len(kchunks) + i][:, :w],
                            bk_sb[:, jc : jc + 1].to_broadcast((128, w)),
                            ALU.add,
                        )

                # ---- V natural projection (+ per-head 64-wide ones blocks) --
                for kt in range(kt_tiles):
                    psv = pp.tile([128, J], F32, tag=f"psq{kt % 2}", name="psv")
                    for dc in range(DC):
                        nc.tensor.matmul(
                            psv[:],
                            xv_sb[:, dc, kt * 128 : (kt + 1) * 128],
                            wv_sb[:, dc, :],
                            start=(dc == 0),
                            stop=(dc == DC - 1),
                        )
                    vt = v_sb[kt]
                    kcol = kones_sb[:, kt : kt + 1]
                    for h in range(HL):
                        pair = h // 2
                        if h % 2 == 0:
                            d0 = pair * 256
                            o0 = pair * 256 + 64
                        else:
                            o0 = pair * 256 + 128
                            d0 = pair * 256 + 192
                        nc.vector.tensor_tensor(
                            vt[:, d0 : d0 + DK],
                            psv[:, h * DK : (h + 1) * DK],
                            bv_bc[:, h * DK : (h + 1) * DK],
                            ALU.add,
                        )
                        nc.vector.tensor_scalar(
                            vt[:, d0 : d0 + DK],
                            vt[:, d0 : d0 + DK],
                            kcol,
                            None,
                            ALU.mult,
                        )
                        nc.vector.tensor_copy(
                            vt[:, o0 : o0 + DK], kcol.to_broadcast((128, DK))
                        )

            # ---- attention + per-granule output projection ------------------
            with (
                tc.tile_pool(name="score_psum", bufs=2, space="PSUM") as sp,
                tc.tile_pool(name="aug_psum", bufs=4, space="PSUM") as ap,
            ):
                for qg in range(NQG):
                    q0 = qg * QG
                    for pair in range(JC):
                        aug_e = ap.tile([128, QG], F32, tag="aug", name="aug_e")
                        aug_o = ap.tile([128, QG], F32, tag="aug", name="aug_o")
                        for kt in range(kt_tiles):
                            ps = sp.tile([128, 1024], F32, tag="ps", name="ps")
                            ksl = slice(kt * 128, (kt + 1) * 128)
                            # row-tiled concurrent score pair (K=64 each)
                            nc.tensor.matmul(
                                ps[:, 0:QG],
                                kt2[pair][0:64, ksl],
                                qt_sb[pair][0:64, q0 : q0 + QG],
                                start=True,
                                stop=True,
                            )
                            nc.tensor.matmul(
                                ps[:, QG : 2 * QG],
                                kt2[pair][64:128, ksl],
                                qt_sb[pair][64:128, q0 : q0 + QG],
                                start=True,
                                stop=True,
                            )
                            pt = ppool.tile([128, 1024], BF16, tag="pt")
                            nc.scalar.activation(pt[:], ps[:], AF.Exp, scale=0.125)
                            nc.tensor.matmul(
                                aug_e[:],
                                v_sb[kt][:, pair * 256 : pair * 256 + 128],
                                pt[:, 0:QG],
                                start=(kt == 0),
                                stop=(kt == kt_tiles - 1),
                            )
                            nc.tensor.matmul(
                                aug_o[:],
                                v_sb[kt][:, pair * 256 + 128 : pair * 256 + 256],
                                pt[:, QG : 2 * QG],
                                start=(kt == 0),
                                stop=(kt == kt_tiles - 1),
                            )
                        # normalize: the custom reciprocal op only works at
                        # base partition 0 on HW, and cross-half DVE moves
                        # are invalid — partition shifts go through small
                        # SBUF->SBUF DMAs instead.
                        de = small.tile([128, QG], F32, tag="de")
                        nc.vector.tensor_copy(de[64:128, :], aug_e[64:128, :])
                        dl = small.tile([128, QG], F32, tag="dl")
                        nc.sync.dma_start(dl[0:64, :], de[64:128, :])
                        rr = small.tile([128, QG], F32, tag="rr")
                        nc.vector.reciprocal_approx_fast(rr[0:64, :], dl[0:64, :])
                        ro = small.tile([128, QG], F32, tag="ro")
                        nc.vector.reciprocal_approx_fast(ro[0:64, :], aug_o[0:64, :])
                        rb = small.tile([128, QG], F32, tag="rb")
                        nc.sync.dma_start(rb[0:64, :], rr[0:64, :])
                        nc.sync.dma_start(rb[64:128, :], ro[0:64, :])
                        nc.vector.tensor_tensor(
                            at_sb[pair][0:64, q0 : q0 + QG],
                            aug_e[0:64, :],
                            rb[0:64, :],
                            ALU.mult,
                        )
                        nc.vector.tensor_tensor(
                            at_sb[pair][64:128, q0 : q0 + QG],
                            aug_o[64:128, :],
                            rb[64:128, :],
                            ALU.mult,
                        )

                    # output projection, lagged one granule so its psum
                    # traffic and normalize latency hide under the next
                    # granule's (scalar-bound) attention work
                    if qg > 0:
                        emit_outproj(nc, sp, small, at_sb, wo_sb, y, qg - 1)
                emit_outproj(nc, sp, small, at_sb, wo_sb, y, NQG - 1)

    nc.finalize()
    return nc


_CACHE: dict = {}


def _get_program(kt_tiles: int):
    if kt_tiles not in _CACHE:
        _CACHE[kt_tiles] = build_program(kt_tiles)
    return _CACHE[kt_tiles]


def _prep_inputs(q, k, v, mask, Wq, bq, Wk, bk, Wv, bv, Wo, bo):
    """Shard + transpose + compact on the host. Returns (in_maps, kt_tiles)."""
    idx = [np.nonzero(mask[b])[0] for b in range(B)]
    s_u = max(1, max(len(i) for i in idx))
    kt_tiles = (s_u + 127) // 128
    k_pad = kt_tiles * 128

    per_batch = []
    for b in range(B):
        qT = np.ascontiguousarray(q[b].T).astype(BF16NP)  # [D, S]
        kT = np.zeros((D, k_pad), BF16NP)
        vT = np.zeros((D, k_pad), BF16NP)
        n = len(idx[b])
        kT[:, :n] = k[b].T[:, idx[b]].astype(BF16NP)
        vT[:, :n] = v[b].T[:, idx[b]].astype(BF16NP)
        ko = np.zeros((k_pad,), np.float32)
        ko[:n] = 1.0
        per_batch.append((qT, kT, vT, ko))

    in_maps = []
    for core in range(NCORES):
        b, g = divmod(core, GROUPS)
        j0 = g * J
        qT, kT, vT, ko = per_batch[b]
        in_maps.append(
            {
                "xq": qT,
                "xk": kT,
                "xv": vT,
                "wq": np.ascontiguousarray(Wq[j0 : j0 + J, :].T).astype(BF16NP),
                "wk": np.ascontiguousarray(Wk[j0 : j0 + J, :].T).astype(BF16NP),
                "wv": np.ascontiguousarray(Wv[j0 : j0 + J, :].T).astype(BF16NP),
                "wo": np.ascontiguousarray(Wo[:, j0 : j0 + J].T).astype(BF16NP),
                "bq": np.ascontiguousarray(bq[j0 : j0 + J]).astype(np.float32),
                "bk": np.ascontiguousarray(bk[j0 : j0 + J]).astype(np.float32),
                "bv": np.ascontiguousarray(bv[j0 : j0 + J]).astype(np.float32),
                "kones": ko,
            }
        )
    return in_maps, kt_tiles


def run(inputs: dict, trace: bool = False):
    """Run the sharded kernel; returns (output [B,S,D] f32, BassKernelResults)."""
    from concourse.bass_utils import run_bass_kernel_spmd

    inputs = {k: np.asarray(v) for k, v in inputs.items()}
    in_maps, kt_tiles = _prep_inputs(**inputs)
    nc = _get_program(kt_tiles)
    res = run_bass_kernel_spmd(nc, in_maps, list(range(NCORES)), trace=trace)
    bo = inputs["bo"].astype(np.float32)
    out = np.empty((B, S, D), np.float32)
    for b in range(B):
        acc = np.zeros((S, D), np.float64)
        for g in range(GROUPS):
            acc += np.asarray(res.results[b * GROUPS + g]["y"], dtype=np.float64)
        out[b] = (acc + bo[None, :]).astype(np.float32)
    return out, res


def kernel(**inputs) -> np.ndarray:
    out, _ = run(inputs, trace=False)
    return out
